# revision 4
# baseline (speedup 1.0000x reference)
"""Trainium2 Bass kernel for nn_Attention (dense transformer attention block).

Reference computation (shapes fixed):
  x [2, 256, 48, 48] -> RMSNorm over channels -> 1x1 conv to qkv (8 heads, 64 dhead)
  -> prepend 4 learnable mem kv tokens -> softmax attention -> 1x1 conv out [2, 256, 48, 48]

Sharding: 8 cores = 2 batches x 4 head-pairs. Core c handles batch c//4 and
heads (2g, 2g+1) where g = c%4. Each core computes its heads' attention and a
partial output projection [256, 2304]; partials are AllReduced within each
batch's 4-core group; host takes one copy per batch.

All matmuls run in float32r (full-rate PE). Layout highlights:
  - x, xn kept [channel, pos]; RMSNorm scale via all-ones-lhsT matmul that
    broadcasts sum-of-squares to all partitions.
  - q/k/v in [dhead(2 heads packed), pos]; sim matmuls row-packed (head A on
    PE rows 0-63, head B on 64-127, concurrent).
  - scores S^T [key, query] in psum; exp on ACT straight psum->sbuf; P @ v^T
    accumulated in psum with lhsT columns [ones | zeros*63 | v], giving the
    softmax denominator on partition 0 and out^T on partitions 64-127.
  - denominator: reciprocal (DVE) + partition broadcast (GpSimd).
"""
import numpy as np

import concourse.mybir as mybir
import concourse.tile as tile
from concourse import bacc
from concourse.bass_utils import run_bass_kernel_spmd
from concourse.masks import make_identity

F32 = mybir.dt.float32
F32R = mybir.dt.float32r
EXP = mybir.ActivationFunctionType.Exp
SQRT = mybir.ActivationFunctionType.Sqrt
SQUARE = mybir.ActivationFunctionType.Square

DIM = 256
HEADS = 8
DHEAD = 64
MEM = 4
HID = 512
N = 48 * 48          # 2304 image positions
NK = N + MEM         # 2308 keys (mem tokens at the END: cols 2304:2308)
NJT = N // 128       # 18 image j-tiles
GROUPS = [[0, 1, 2, 3], [4, 5, 6, 7]]

# i-chunks of the query axis
CHUNKS = [(0, 512), (512, 512), (1024, 512), (1536, 512), (2048, 256)]


def build():
    nc = bacc.Bacc("TRN2", target_bir_lowering=False, debug=False,
                   enable_asserts=True, num_devices=8)
    x_d = nc.dram_tensor("x", [DIM, N], F32, kind="ExternalInput").ap()
    wqkv_d = nc.dram_tensor("wqkv", [DIM, 384], F32, kind="ExternalInput").ap()
    memk_d = nc.dram_tensor("memk", [128, MEM], F32, kind="ExternalInput").ap()
    memv_d = nc.dram_tensor("memv", [MEM, 2, DHEAD], F32, kind="ExternalInput").ap()
    woutT_d = nc.dram_tensor("woutT", [2, DHEAD, DIM], F32, kind="ExternalInput").ap()
    out_d = nc.dram_tensor("out", [DIM, N], F32, kind="ExternalOutput").ap()

    with tile.TileContext(nc) as tc:
        with (
            tc.tile_pool(name="consts", bufs=1) as consts,
            tc.tile_pool(name="big", bufs=1) as big,
            tc.tile_pool(name="io", bufs=2) as io,
            tc.tile_pool(name="pP", bufs=3) as pP,
            tc.tile_pool(name="ps_s", bufs=2, space="PSUM") as ps_s,
            tc.tile_pool(name="ps_acc", bufs=1, space="PSUM") as ps_acc,
            tc.tile_pool(name="ps_o", bufs=2, space="PSUM") as ps_o,
        ):
            # ---------------- constants ----------------
            ident_f = consts.tile([128, 128], F32)
            make_identity(nc, ident_f)
            ident = consts.tile([128, 128], F32R)
            nc.vector.tensor_copy(ident[:, :], ident_f[:, :])
            ones_f = consts.tile([128, 1], F32)
            nc.vector.memset(ones_f[:, :], 1.0)
            zeros_f = consts.tile([128, 1], F32)
            nc.vector.memset(zeros_f[:, :], 0.0)
            ones_r = consts.tile([128, 128], F32R)
            nc.vector.tensor_copy(ones_r[:, :], ones_f[:, :].to_broadcast((128, 128)))

            # ---------------- load inputs ----------------
            x0 = big.tile([128, N], F32)
            x1 = big.tile([128, N], F32)
            nc.sync.dma_start(out=x0[:, :], in_=x_d[0:128, :])
            nc.sync.dma_start(out=x1[:, :], in_=x_d[128:256, :])

            wq_f = io.tile([128, 2, 384], F32)
            nc.sync.dma_start(out=wq_f[:, 0, :], in_=wqkv_d[0:128, :])
            nc.sync.dma_start(out=wq_f[:, 1, :], in_=wqkv_d[128:256, :])
            wq = consts.tile([128, 2, 384], F32R)
            nc.vector.tensor_copy(wq[:, :, :], wq_f[:, :, :])

            memk_f = io.tile([128, MEM], F32)
            nc.sync.dma_start(out=memk_f[:, :], in_=memk_d)
            memv_f = io.tile([MEM, 2, DHEAD], F32)
            nc.sync.dma_start(out=memv_f[:, :, :], in_=memv_d)

            # wout lhsT tiles, one per head, data on partitions 64..127
            woutA_f = io.tile([128, DIM], F32, tag="woutA_f")
            woutB_f = io.tile([128, DIM], F32, tag="woutB_f")
            nc.sync.dma_start(out=woutA_f[64:128, :], in_=woutT_d[0, :, :])
            nc.sync.dma_start(out=woutB_f[64:128, :], in_=woutT_d[1, :, :])
            woutA = consts.tile([128, DIM], F32R, tag="woutA")
            woutB = consts.tile([128, DIM], F32R, tag="woutB")
            nc.vector.tensor_copy(woutA[64:128, :], woutA_f[64:128, :])
            nc.vector.tensor_copy(woutB[64:128, :], woutB_f[64:128, :])
            wouts = [woutA, woutB]

            # ---------------- RMSNorm ----------------
            # ssq broadcast to all partitions via all-ones lhsT
            xsq0 = big.tile([128, N], F32R, tag="xsq0")
            xsq1 = big.tile([128, N], F32R, tag="xsq1")
            nc.scalar.activation(xsq0[:, :], x0[:, :], SQUARE)
            nc.scalar.activation(xsq1[:, :], x1[:, :], SQUARE)
            sinv = big.tile([128, N], F32)
            for c0, cw in [(0, 1024), (1024, 1024), (2048, 256)]:
                sb_ps = ps_s.tile([128, 1024], F32, tag="s")
                for n0 in range(0, cw, 512):
                    nw = min(512, cw - n0)
                    nc.tensor.matmul(
                        sb_ps[:, n0:n0 + nw],
                        ones_r[:, :], xsq0[:, c0 + n0:c0 + n0 + nw],
                        start=True, stop=False,
                    )
                    nc.tensor.matmul(
                        sb_ps[:, n0:n0 + nw],
                        ones_r[:, :], xsq1[:, c0 + n0:c0 + n0 + nw],
                        start=False, stop=True,
                    )
                # sqrt(ssq/256) then reciprocal -> 16/sqrt(ssq)
                nc.scalar.activation(sinv[:, c0:c0 + cw], sb_ps[:, 0:cw], SQRT,
                                     scale=1.0 / 256.0)
            nc.vector.reciprocal_approx_fast(sinv[:, :], sinv[:, :])
            xn0 = big.tile([128, N], F32R, tag="xn0")
            xn1 = big.tile([128, N], F32R, tag="xn1")
            nc.vector.tensor_mul(xn0[:, :], x0[:, :], sinv[:, :])
            nc.vector.tensor_mul(xn1[:, :], x1[:, :], sinv[:, :])
            xns = [xn0, xn1]

            # ---------------- qkv projection ----------------
            q_sb = big.tile([128, N], F32R)
            k_sb = big.tile([128, NK], F32R)
            v_sb = big.tile([128, N], F32R)
            dsts = [q_sb, k_sb, v_sb]
            for m in range(3):
                for c0, cw in CHUNKS:
                    qp = ps_o.tile([128, 512], F32, tag="o")
                    for kt in range(2):
                        nc.tensor.matmul(
                            qp[:, 0:cw],
                            wq[:, kt, m * 128:(m + 1) * 128],
                            xns[kt][:, c0:c0 + cw],
                            start=(kt == 0), stop=(kt == 1),
                        )
                    nc.vector.tensor_copy(dsts[m][:, c0:c0 + cw], qp[:, 0:cw])
            # mem keys at k columns 2304:2308
            nc.vector.tensor_copy(k_sb[:, N:NK], memk_f[:, :])

            # ---------------- v^T tiles ----------------
            # per head: [key(128 part), 19 jt, 128]: col 0 ones, 1:64 zeros, 64:128 v
            vTA = big.tile([128, NJT + 1, 128], F32R, tag="vTA")
            vTB = big.tile([128, NJT + 1, 128], F32R, tag="vTB")
            vTs = [vTA, vTB]
            for h in range(2):
                nc.vector.tensor_copy(
                    vTs[h][:, :, 0:1],
                    ones_f[:, :].to_broadcast((128, NJT + 1, 1)))
                nc.vector.tensor_copy(
                    vTs[h][:, :, 1:64],
                    zeros_f[:, :].to_broadcast((128, NJT + 1, 63)))
                for jt in range(NJT):
                    tp = ps_o.tile([128, 64], F32R, tag="o")
                    nc.tensor.transpose(
                        tp[:, :],
                        v_sb[64 * h:64 * h + 64, jt * 128:(jt + 1) * 128],
                        ident[64 * h:64 * h + 64, 64 * h:64 * h + 64],
                    )
                    nc.vector.tensor_copy(vTs[h][:, jt, 64:128], tp[:, :])
                # mem values into the last j-tile (rows 0:4)
                nc.vector.tensor_copy(vTs[h][0:MEM, NJT, 64:128], memv_f[:, h, :])

            # ---------------- attention + out projection ----------------
            osb0 = big.tile([128, N], F32, tag="osb0")
            osb1 = big.tile([128, N], F32, tag="osb1")
            osbs = [osb0, osb1]
            rec = io.tile([1, 2, 512], F32, tag="rec")
            for ci, (c0, cw) in enumerate(CHUNKS):
                acc0 = ps_acc.tile([128, 512], F32, tag="acc0")
                acc1 = ps_acc.tile([128, 512], F32, tag="acc1")
                accs = [acc0, acc1]
                for jt in range(NJT + 1):
                    if jt < NJT:
                        kk, km = jt * 128, 128
                    else:
                        kk, km = N, MEM
                    s_ps = ps_s.tile([128, 2, 512], F32, tag="s")
                    for h in range(2):
                        nc.tensor.matmul(
                            s_ps[0:km, h, 0:cw],
                            k_sb[64 * h:64 * h + 64, kk:kk + km],
                            q_sb[64 * h:64 * h + 64, c0:c0 + cw],
                            start=True, stop=True,
                        )
                    P = pP.tile([128, 2, 512], F32R, tag="P")
                    nc.scalar.activation(P[0:km, :, 0:cw], s_ps[0:km, :, 0:cw], EXP)
                    for h in range(2):
                        nc.tensor.matmul(
                            accs[h][:, 0:cw],
                            vTs[h][0:km, jt, :],
                            P[0:km, h, 0:cw],
                            start=(jt == 0), stop=(jt == NJT),
                            skip_group_check=True,
                        )
                # normalize: out^T_h = acc[64:128] * (1/acc[0])
                rb = pP.tile([128, 2, 512], F32, tag="rb")
                for h in range(2):
                    nc.vector.reciprocal(rec[0:1, h, 0:cw], accs[h][0:1, 0:cw])
                    nc.gpsimd.partition_broadcast(rb[:, h, 0:cw], rec[0:1, h, 0:cw])
                oT0 = pP.tile([128, 512], F32R, tag="oT0")
                oT1 = pP.tile([128, 512], F32R, tag="oT1")
                oTs = [oT0, oT1]
                for h in range(2):
                    nc.vector.tensor_mul(
                        oTs[h][64:128, 0:cw], accs[h][64:128, 0:cw],
                        rb[64:128, h, 0:cw])
                # out projection: [256, cw] partial = sum_h woutT_h.T @ oT_h
                for mt in range(2):
                    op = ps_o.tile([128, 512], F32, tag="o")
                    for h in range(2):
                        nc.tensor.matmul(
                            op[:, 0:cw],
                            wouts[h][64:128, mt * 128:(mt + 1) * 128],
                            oTs[h][64:128, 0:cw],
                            start=(h == 0), stop=(h == 1),
                        )
                    nc.vector.tensor_copy(osbs[mt][:, c0:c0 + cw], op[:, 0:cw])

            # ---------------- reduce across the 4 cores of this batch ----------
            with tc.tile_pool(name="dram", bufs=1, space="DRAM") as dram:
                bounce_in = dram.tile([DIM, N], F32)
                bounce_out = dram.tile([DIM, N], F32)
                nc.sync.dma_start(out=bounce_in[0:128, :], in_=osb0[:, :])
                nc.sync.dma_start(out=bounce_in[128:256, :], in_=osb1[:, :])
                nc.gpsimd.collective_compute(
                    "AllReduce", mybir.AluOpType.add,
                    replica_groups=GROUPS,
                    ins=[bounce_in[:, :].opt()],
                    outs=[bounce_out[:, :].opt()],
                )
                nc.sync.dma_start(out=out_d, in_=bounce_out[:, :])
    nc.compile()
    return nc


_NC = None


def _get_nc():
    global _NC
    if _NC is None:
        _NC = build()
    return _NC


def kernel(x, gamma, mem_kv, w_qkv, w_out):
    x = np.asarray(x, np.float32)
    gamma = np.asarray(gamma, np.float32).reshape(DIM)
    mem_kv = np.asarray(mem_kv, np.float32)
    w_qkv = np.asarray(w_qkv, np.float32)
    w_out = np.asarray(w_out, np.float32)

    g1 = 1.0 + gamma  # [256]
    scale = DHEAD ** -0.5
    in_maps = []
    for core in range(8):
        b, g = core // 4, core % 4
        hA, hB = 2 * g, 2 * g + 1
        blocks = []
        for t in range(3):  # q, k, v
            for h in (hA, hB):
                wblk = w_qkv[t * HID + h * DHEAD: t * HID + (h + 1) * DHEAD, :]
                if t == 0:
                    wblk = wblk * scale
                blocks.append(wblk.T)  # [256, 64]
        wqkvT = np.concatenate(blocks, axis=1) * g1[:, None]  # [256, 384]
        memk = np.concatenate(
            [mem_kv[0, hA].T, mem_kv[0, hB].T], axis=0)  # [128, 4]
        memv = np.stack([mem_kv[1, hA], mem_kv[1, hB]], axis=1)  # [4, 2, 64]
        woutT = np.stack(
            [w_out[:, hA * DHEAD:(hA + 1) * DHEAD].T,
             w_out[:, hB * DHEAD:(hB + 1) * DHEAD].T], axis=0)  # [2, 64, 256]
        in_maps.append({
            "x": np.ascontiguousarray(x[b].reshape(DIM, N)),
            "wqkv": np.ascontiguousarray(wqkvT),
            "memk": np.ascontiguousarray(memk),
            "memv": np.ascontiguousarray(memv),
            "woutT": np.ascontiguousarray(woutT),
        })

    global _last_in_maps
    _last_in_maps = in_maps
    nc = _get_nc()
    res = run_bass_kernel_spmd(nc, in_maps, core_ids=list(range(8)))
    out = np.stack([res.results[0]["out"], res.results[4]["out"]])
    return out.reshape(2, DIM, 48, 48)


# revision 6
# speedup vs baseline: 1.3145x; 1.3145x over previous
"""Trainium2 Bass kernel for nn_Attention (dense transformer attention block).

Reference computation (shapes fixed):
  x [2, 256, 48, 48] -> RMSNorm over channels -> 1x1 conv to qkv (8 heads, 64 dhead)
  -> prepend 4 learnable mem kv tokens -> softmax attention -> 1x1 conv out [2, 256, 48, 48]

Sharding: 8 cores = 2 batches x 4 head-pairs. Core c handles batch c//4 and
heads (2g, 2g+1), g = c%4. Each core computes its heads' attention and a
partial out-projection [256, 2304]; partials are ReduceScattered (chunked,
overlapped with compute) within each batch's 4-core group; each core returns
its 64-channel slice of the reduced output and the host reassembles.

Numerics: qkv projection in float32r (full-rate PE, ~19-bit); attention
matmuls (sim, attn@v, out-projection) in bf16 with fp32 psum accumulation.
Layout highlights:
  - x, xn in [channel, pos]; RMSNorm scale via all-ones-lhsT matmul that
    broadcasts the sum of squares to all 128 partitions.
  - q/k/v in [dhead(2 heads packed), pos]; sim matmuls row-packed (head A on
    PE rows 0-63, head B on 64-127, concurrent via row groups).
  - scores S^T [key, query] in psum; exp on ACT straight psum->sbuf; P @ v^T
    accumulated in psum with lhsT columns [ones | zeros*63 | v], giving the
    softmax denominator on partition 0 and out^T on partitions 64-127.
  - denominator: fast reciprocal (DVE) + partition broadcast (GpSimd).
"""
import numpy as np

import concourse.mybir as mybir
import concourse.tile as tile
from concourse import bacc
from concourse.bass_utils import run_bass_kernel_spmd
from concourse.masks import make_identity

F32 = mybir.dt.float32
F32R = mybir.dt.float32r
BF16 = mybir.dt.bfloat16
EXP = mybir.ActivationFunctionType.Exp
SQRT = mybir.ActivationFunctionType.Sqrt

DIM = 256
HEADS = 8
DHEAD = 64
MEM = 4
HID = 512
N = 48 * 48          # 2304 image positions
NK = N + MEM         # 2308 keys (mem tokens at the END: cols 2304:2308)
NJT = N // 128       # 18 image j-tiles
GROUPS = [[0, 1, 2, 3], [4, 5, 6, 7]]

# i-chunks of the query axis
CHUNKS = [(0, 512), (512, 512), (1024, 512), (1536, 512), (2048, 256)]


def build():
    nc = bacc.Bacc("TRN2", target_bir_lowering=False, debug=False,
                   enable_asserts=True, num_devices=8)
    x_d = nc.dram_tensor("x", [DIM, N], F32, kind="ExternalInput").ap()
    wqkv_d = nc.dram_tensor("wqkv", [DIM, 384], F32, kind="ExternalInput").ap()
    memk_d = nc.dram_tensor("memk", [128, MEM], F32, kind="ExternalInput").ap()
    memv_d = nc.dram_tensor("memv", [MEM, 2, DHEAD], F32, kind="ExternalInput").ap()
    woutT_d = nc.dram_tensor("woutT", [2, DHEAD, DIM], F32, kind="ExternalInput").ap()
    out_d = nc.dram_tensor("out", [DHEAD, N], F32, kind="ExternalOutput").ap()

    with tile.TileContext(nc) as tc:
        with (
            tc.tile_pool(name="consts", bufs=1) as consts,
            tc.tile_pool(name="big", bufs=1) as big,
            tc.tile_pool(name="io", bufs=2) as io,
            tc.tile_pool(name="pP", bufs=3) as pP,
            tc.tile_pool(name="ps_s", bufs=2, space="PSUM") as ps_s,
            tc.tile_pool(name="ps_a", bufs=2, space="PSUM") as ps_a,
            tc.tile_pool(name="dram", bufs=1, space="DRAM") as dram,
        ):
            # ---------------- constants ----------------
            ident = consts.tile([128, 128], F32)
            make_identity(nc, ident)
            ones_f = consts.tile([128, 1], F32)
            nc.vector.memset(ones_f[:, :], 1.0)
            zeros_f = consts.tile([128, 1], F32)
            nc.vector.memset(zeros_f[:, :], 0.0)
            ones_r = consts.tile([128, 128], F32R)
            nc.vector.tensor_copy(ones_r[:, :], ones_f[:, :].to_broadcast((128, 128)))

            # ---------------- load inputs ----------------
            x0 = big.tile([128, N], F32)
            x1 = big.tile([128, N], F32)
            nc.sync.dma_start(out=x0[:, :], in_=x_d[0:128, :])
            nc.sync.dma_start(out=x1[:, :], in_=x_d[128:256, :])

            wq_f = io.tile([128, 2, 384], F32)
            nc.sync.dma_start(out=wq_f[:, 0, :], in_=wqkv_d[0:128, :])
            nc.sync.dma_start(out=wq_f[:, 1, :], in_=wqkv_d[128:256, :])
            wq = consts.tile([128, 2, 384], F32R)
            nc.vector.tensor_copy(wq[:, :, :], wq_f[:, :, :])

            memk_f = io.tile([128, MEM], F32)
            nc.sync.dma_start(out=memk_f[:, :], in_=memk_d)
            memv_f = io.tile([MEM, 2, DHEAD], F32)
            nc.sync.dma_start(out=memv_f[:, :, :], in_=memv_d)

            # wout lhsT tiles, one per head, data on partitions 64..127
            woutA_f = io.tile([128, DIM], F32, tag="woutA_f")
            woutB_f = io.tile([128, DIM], F32, tag="woutB_f")
            nc.sync.dma_start(out=woutA_f[64:128, :], in_=woutT_d[0, :, :])
            nc.sync.dma_start(out=woutB_f[64:128, :], in_=woutT_d[1, :, :])
            woutA = consts.tile([128, DIM], BF16, tag="woutA")
            woutB = consts.tile([128, DIM], BF16, tag="woutB")
            nc.vector.tensor_copy(woutA[64:128, :], woutA_f[64:128, :])
            nc.vector.tensor_copy(woutB[64:128, :], woutB_f[64:128, :])
            wouts = [woutA, woutB]

            # ---------------- RMSNorm ----------------
            xsq0 = big.tile([128, N], F32R, tag="xsq0")
            xsq1 = big.tile([128, N], F32R, tag="xsq1")
            nc.vector.tensor_mul(xsq0[:, :], x0[:, :], x0[:, :])
            nc.vector.tensor_mul(xsq1[:, :], x1[:, :], x1[:, :])
            sinv = big.tile([128, N], F32)
            for c0, cw in [(0, 1024), (1024, 1024), (2048, 256)]:
                sb_ps = ps_s.tile([128, 1024], F32, tag="s")
                for n0 in range(0, cw, 512):
                    nw = min(512, cw - n0)
                    nc.tensor.matmul(
                        sb_ps[:, n0:n0 + nw],
                        ones_r[:, :], xsq0[:, c0 + n0:c0 + n0 + nw],
                        start=True, stop=False,
                    )
                    nc.tensor.matmul(
                        sb_ps[:, n0:n0 + nw],
                        ones_r[:, :], xsq1[:, c0 + n0:c0 + n0 + nw],
                        start=False, stop=True,
                    )
                # sqrt(ssq/256) then reciprocal -> 16/sqrt(ssq)
                nc.scalar.activation(sinv[:, c0:c0 + cw], sb_ps[:, 0:cw], SQRT,
                                     scale=1.0 / 256.0)
            nc.vector.reciprocal_approx_fast(sinv[:, :], sinv[:, :])
            xn0 = big.tile([128, N], F32R, tag="xn0")
            xn1 = big.tile([128, N], F32R, tag="xn1")
            nc.vector.tensor_mul(xn0[:, :], x0[:, :], sinv[:, :])
            nc.vector.tensor_mul(xn1[:, :], x1[:, :], sinv[:, :])
            xns = [xn0, xn1]

            # ---------------- qkv projection (f32r) ----------------
            q_sb = big.tile([128, N], BF16)
            k_sb = big.tile([128, NK], BF16)
            v_sb = big.tile([128, N], F32)
            dsts = [q_sb, k_sb, v_sb]
            for m in range(3):
                for c0, cw in CHUNKS:
                    qp = ps_a.tile([128, 512], F32, tag="a0")
                    for kt in range(2):
                        nc.tensor.matmul(
                            qp[:, 0:cw],
                            wq[:, kt, m * 128:(m + 1) * 128],
                            xns[kt][:, c0:c0 + cw],
                            start=(kt == 0), stop=(kt == 1),
                        )
                    nc.vector.tensor_copy(dsts[m][:, c0:c0 + cw], qp[:, 0:cw])
            # mem keys at k columns 2304:2308
            nc.vector.tensor_copy(k_sb[:, N:NK], memk_f[:, :])

            # ---------------- v^T tiles ----------------
            # per head: [key(128 part), 19 jt, 128]: col 0 ones, 1:64 zeros, 64:128 v
            vTA = big.tile([128, NJT + 1, 128], BF16, tag="vTA")
            vTB = big.tile([128, NJT + 1, 128], BF16, tag="vTB")
            vTs = [vTA, vTB]
            for h in range(2):
                nc.vector.tensor_copy(
                    vTs[h][:, :, 0:1],
                    ones_f[:, :].to_broadcast((128, NJT + 1, 1)))
                nc.vector.tensor_copy(
                    vTs[h][:, :, 1:64],
                    zeros_f[:, :].to_broadcast((128, NJT + 1, 63)))
                for jt in range(NJT):
                    tp = ps_a.tile([128, 64], F32, tag="a1")
                    nc.tensor.transpose(
                        tp[:, :],
                        v_sb[64 * h:64 * h + 64, jt * 128:(jt + 1) * 128],
                        ident[64 * h:64 * h + 64, 64 * h:64 * h + 64],
                    )
                    nc.vector.tensor_copy(vTs[h][:, jt, 64:128], tp[:, :])
                # mem values into the last j-tile (rows 0:4)
                nc.vector.tensor_copy(vTs[h][0:MEM, NJT, 64:128], memv_f[:, h, :])

            # ---------------- attention + out projection ----------------
            rec = io.tile([1, 2, 512], F32, tag="rec")
            for ci, (c0, cw) in enumerate(CHUNKS):
                acc0 = ps_a.tile([128, 512], F32, tag="a0")
                acc1 = ps_a.tile([128, 512], F32, tag="a1")
                accs = [acc0, acc1]
                for jt in range(NJT + 1):
                    if jt < NJT:
                        kk, km = jt * 128, 128
                    else:
                        kk, km = N, MEM
                    s_ps = ps_s.tile([128, 2, 512], F32, tag="s")
                    for h in range(2):
                        nc.tensor.matmul(
                            s_ps[0:km, h, 0:cw],
                            k_sb[64 * h:64 * h + 64, kk:kk + km],
                            q_sb[64 * h:64 * h + 64, c0:c0 + cw],
                            start=True, stop=True,
                        )
                    P = pP.tile([128, 2, 512], BF16, tag="P")
                    nc.scalar.activation(P[0:km, :, 0:cw], s_ps[0:km, :, 0:cw], EXP)
                    for h in range(2):
                        nc.tensor.matmul(
                            accs[h][:, 0:cw],
                            vTs[h][0:km, jt, :],
                            P[0:km, h, 0:cw],
                            start=(jt == 0), stop=(jt == NJT),
                            skip_group_check=True,
                        )
                # normalize: out^T_h = acc[64:128] * (1/acc[0])
                rb = pP.tile([128, 2, 512], F32, tag="rb")
                for h in range(2):
                    nc.vector.reciprocal_approx_fast(
                        rec[0:1, h, 0:cw], accs[h][0:1, 0:cw])
                    nc.gpsimd.partition_broadcast(rb[:, h, 0:cw], rec[0:1, h, 0:cw])
                oT0 = pP.tile([128, 512], BF16, tag="oT0")
                oT1 = pP.tile([128, 512], BF16, tag="oT1")
                oTs = [oT0, oT1]
                for h in range(2):
                    nc.vector.tensor_mul(
                        oTs[h][64:128, 0:cw], accs[h][64:128, 0:cw],
                        rb[64:128, h, 0:cw])
                # out projection: [256, cw] partial = sum_h woutT_h.T @ oT_h
                osb = pP.tile([128, 2, 512], F32, tag="osb")
                for mt in range(2):
                    op = ps_a.tile([128, 512], F32, tag=f"a{mt}")
                    for h in range(2):
                        nc.tensor.matmul(
                            op[:, 0:cw],
                            wouts[h][64:128, mt * 128:(mt + 1) * 128],
                            oTs[h][64:128, 0:cw],
                            start=(h == 0), stop=(h == 1),
                        )
                    nc.vector.tensor_copy(osb[:, mt, 0:cw], op[:, 0:cw])
                # chunked reduce-scatter of the [256, cw] partial
                bi = dram.tile([2, 128, cw], F32, tag=f"bi{ci}")
                bo = dram.tile([DHEAD, cw], F32, tag=f"bo{ci}")
                nc.sync.dma_start(out=bi[0, :, :], in_=osb[:, 0, 0:cw])
                nc.sync.dma_start(out=bi[1, :, :], in_=osb[:, 1, 0:cw])
                nc.gpsimd.collective_compute(
                    "ReduceScatter", mybir.AluOpType.add,
                    replica_groups=GROUPS,
                    ins=[bi[:, :, :].opt()],
                    outs=[bo[:, :].opt()],
                )
                nc.sync.dma_start(out=out_d[:, c0:c0 + cw], in_=bo[:, :])
    nc.compile()
    return nc


_NC = None
_last_in_maps = None


def _get_nc():
    global _NC
    if _NC is None:
        _NC = build()
    return _NC


def make_in_maps(x, gamma, mem_kv, w_qkv, w_out):
    x = np.asarray(x, np.float32)
    gamma = np.asarray(gamma, np.float32).reshape(DIM)
    mem_kv = np.asarray(mem_kv, np.float32)
    w_qkv = np.asarray(w_qkv, np.float32)
    w_out = np.asarray(w_out, np.float32)

    g1 = 1.0 + gamma  # [256]
    scale = DHEAD ** -0.5
    in_maps = []
    for core in range(8):
        b, g = core // 4, core % 4
        hA, hB = 2 * g, 2 * g + 1
        blocks = []
        for t in range(3):  # q, k, v
            for h in (hA, hB):
                wblk = w_qkv[t * HID + h * DHEAD: t * HID + (h + 1) * DHEAD, :]
                if t == 0:
                    wblk = wblk * scale
                blocks.append(wblk.T)  # [256, 64]
        wqkvT = np.concatenate(blocks, axis=1) * g1[:, None]  # [256, 384]
        memk = np.concatenate(
            [mem_kv[0, hA].T, mem_kv[0, hB].T], axis=0)  # [128, 4]
        memv = np.stack([mem_kv[1, hA], mem_kv[1, hB]], axis=1)  # [4, 2, 64]
        woutT = np.stack(
            [w_out[:, hA * DHEAD:(hA + 1) * DHEAD].T,
             w_out[:, hB * DHEAD:(hB + 1) * DHEAD].T], axis=0)  # [2, 64, 256]
        in_maps.append({
            "x": np.ascontiguousarray(x[b].reshape(DIM, N)),
            "wqkv": np.ascontiguousarray(wqkvT),
            "memk": np.ascontiguousarray(memk),
            "memv": np.ascontiguousarray(memv),
            "woutT": np.ascontiguousarray(woutT),
        })
    return in_maps


def kernel(x, gamma, mem_kv, w_qkv, w_out):
    global _last_in_maps
    in_maps = make_in_maps(x, gamma, mem_kv, w_qkv, w_out)
    _last_in_maps = in_maps
    nc = _get_nc()
    res = run_bass_kernel_spmd(nc, in_maps, core_ids=list(range(8)))
    out = np.empty((2, DIM, N), np.float32)
    for core in range(8):
        b, g = core // 4, core % 4
        out[b, 64 * g:64 * g + 64, :] = res.results[core]["out"]
    return out.reshape(2, DIM, 48, 48)


# revision 7
# speedup vs baseline: 1.8233x; 1.3871x over previous
"""Trainium2 Bass kernel for nn_Attention (dense transformer attention block).

Reference computation (shapes fixed):
  x [2, 256, 48, 48] -> RMSNorm over channels -> 1x1 conv to qkv (8 heads, 64 dhead)
  -> prepend 4 learnable mem kv tokens -> softmax attention -> 1x1 conv out [2, 256, 48, 48]

Sharding: 8 cores = 2 batches x 4 head-pairs. Core c handles batch c//4 and
heads (2g, 2g+1), g = c%4. Each core computes its heads' attention and a
partial out-projection [256, 2304]; partials are ReduceScattered (chunked,
overlapped with compute) within each batch's 4-core group; each core returns
its 64-channel slice of the reduced output and the host reassembles.

Numerics: qkv projection in float32r (full-rate PE, ~19-bit); attention
matmuls (sim, attn@v, out-projection) in bf16 with fp32 psum accumulation.
Layout highlights:
  - x, xn in [channel, pos]; RMSNorm scale via all-ones-lhsT matmul that
    broadcasts the sum of squares to all 128 partitions.
  - q/k/v in [dhead(2 heads packed), pos]; sim matmuls row-packed (head A on
    PE rows 0-63, head B on 64-127, concurrent via row groups).
  - scores S^T [key, query] in psum; exp on ACT straight psum->sbuf; P @ v^T
    accumulated in psum with lhsT columns [ones | zeros*63 | v], giving the
    softmax denominator on partition 0 and out^T on partitions 64-127.
  - denominator: fast reciprocal (DVE) + partition broadcast (GpSimd).
"""
import numpy as np

import concourse.mybir as mybir
import concourse.tile as tile
from concourse import bacc
from concourse.bass_utils import run_bass_kernel_spmd
from concourse.masks import make_identity

F32 = mybir.dt.float32
F32R = mybir.dt.float32r
BF16 = mybir.dt.bfloat16
EXP = mybir.ActivationFunctionType.Exp
SQRT = mybir.ActivationFunctionType.Sqrt

DIM = 256
HEADS = 8
DHEAD = 64
MEM = 4
HID = 512
N = 48 * 48          # 2304 image positions
NK = N + MEM         # 2308 keys (mem tokens at the END: cols 2304:2308)
NJT = N // 128       # 18 image j-tiles
GROUPS = [[0, 1, 2, 3], [4, 5, 6, 7]]

# i-chunks of the query axis
CHUNKS = [(0, 512), (512, 512), (1024, 512), (1536, 512), (2048, 256)]


def build():
    nc = bacc.Bacc("TRN2", target_bir_lowering=False, debug=False,
                   enable_asserts=True, num_devices=8)
    x_d = nc.dram_tensor("x", [DIM, N], F32, kind="ExternalInput").ap()
    wqkv_d = nc.dram_tensor("wqkv", [DIM, 384], F32, kind="ExternalInput").ap()
    memk_d = nc.dram_tensor("memk", [128, MEM], F32, kind="ExternalInput").ap()
    memv_d = nc.dram_tensor("memv", [MEM, 2, DHEAD], F32, kind="ExternalInput").ap()
    woutT_d = nc.dram_tensor("woutT", [2, DHEAD, DIM], F32, kind="ExternalInput").ap()
    out_d = nc.dram_tensor("out", [DHEAD, N], F32, kind="ExternalOutput").ap()

    with tile.TileContext(nc) as tc:
        with (
            tc.tile_pool(name="consts", bufs=1) as consts,
            tc.tile_pool(name="big", bufs=1) as big,
            tc.tile_pool(name="io", bufs=2) as io,
            tc.tile_pool(name="pP", bufs=3) as pP,
            tc.tile_pool(name="ps_s", bufs=2, space="PSUM") as ps_s,
            tc.tile_pool(name="ps_a", bufs=2, space="PSUM") as ps_a,
            tc.tile_pool(name="dram", bufs=1, space="DRAM") as dram,
        ):
            # ---------------- constants ----------------
            ident = consts.tile([128, 128], F32)
            make_identity(nc, ident)
            ones_f = consts.tile([128, 1], F32)
            nc.vector.memset(ones_f[:, :], 1.0)
            zeros_f = consts.tile([128, 1], F32)
            nc.vector.memset(zeros_f[:, :], 0.0)
            ones_r = consts.tile([128, 128], F32R)
            nc.vector.tensor_copy(ones_r[:, :], ones_f[:, :].to_broadcast((128, 128)))

            # ---------------- collective warmup ----------------
            # the first collective on a NEFF pays ~60us of firmware cold
            # start; absorb it behind the compute phase with a tiny dummy.
            warm_sb = consts.tile([1, 32], F32)
            nc.vector.memset(warm_sb[:, :], 0.0)
            wi = dram.tile([1, 32], F32, tag="wi")
            wo = dram.tile([1, 32], F32, tag="wo")
            nc.sync.dma_start(out=wi[:, :], in_=warm_sb[:, :])
            nc.gpsimd.collective_compute(
                "AllReduce", mybir.AluOpType.add,
                replica_groups=GROUPS,
                ins=[wi[:, :].opt()],
                outs=[wo[:, :].opt()],
            )

            # ---------------- load inputs ----------------
            x0 = big.tile([128, N], F32)
            x1 = big.tile([128, N], F32)
            nc.sync.dma_start(out=x0[:, :], in_=x_d[0:128, :])
            nc.sync.dma_start(out=x1[:, :], in_=x_d[128:256, :])

            wq_f = io.tile([128, 2, 384], F32)
            nc.sync.dma_start(out=wq_f[:, 0, :], in_=wqkv_d[0:128, :])
            nc.sync.dma_start(out=wq_f[:, 1, :], in_=wqkv_d[128:256, :])
            wq = consts.tile([128, 2, 384], F32R)
            nc.vector.tensor_copy(wq[:, :, :], wq_f[:, :, :])

            memk_f = io.tile([128, MEM], F32)
            nc.sync.dma_start(out=memk_f[:, :], in_=memk_d)
            memv_f = io.tile([MEM, 2, DHEAD], F32)
            nc.sync.dma_start(out=memv_f[:, :, :], in_=memv_d)

            # wout lhsT tiles, one per head, data on partitions 64..127
            woutA_f = io.tile([128, DIM], F32, tag="woutA_f")
            woutB_f = io.tile([128, DIM], F32, tag="woutB_f")
            nc.sync.dma_start(out=woutA_f[64:128, :], in_=woutT_d[0, :, :])
            nc.sync.dma_start(out=woutB_f[64:128, :], in_=woutT_d[1, :, :])
            woutA = consts.tile([128, DIM], BF16, tag="woutA")
            woutB = consts.tile([128, DIM], BF16, tag="woutB")
            nc.vector.tensor_copy(woutA[64:128, :], woutA_f[64:128, :])
            nc.vector.tensor_copy(woutB[64:128, :], woutB_f[64:128, :])
            wouts = [woutA, woutB]

            # ---------------- RMSNorm ----------------
            xsq0 = big.tile([128, N], F32R, tag="xsq0")
            xsq1 = big.tile([128, N], F32R, tag="xsq1")
            SQUARE = mybir.ActivationFunctionType.Square
            nc.scalar.activation(xsq0[:, :], x0[:, :], SQUARE)
            nc.scalar.activation(xsq1[:, :], x1[:, :], SQUARE)
            sinv = big.tile([128, N], F32)
            for c0, cw in [(0, 1024), (1024, 1024), (2048, 256)]:
                sb_ps = ps_s.tile([128, 1024], F32, tag="s")
                for n0 in range(0, cw, 512):
                    nw = min(512, cw - n0)
                    nc.tensor.matmul(
                        sb_ps[:, n0:n0 + nw],
                        ones_r[:, :], xsq0[:, c0 + n0:c0 + n0 + nw],
                        start=True, stop=False,
                    )
                    nc.tensor.matmul(
                        sb_ps[:, n0:n0 + nw],
                        ones_r[:, :], xsq1[:, c0 + n0:c0 + n0 + nw],
                        start=False, stop=True,
                    )
                # sqrt(ssq/256) then reciprocal -> 16/sqrt(ssq)
                nc.scalar.activation(sinv[:, c0:c0 + cw], sb_ps[:, 0:cw], SQRT,
                                     scale=1.0 / 256.0)
            nc.vector.reciprocal_approx_fast(sinv[:, :], sinv[:, :])
            xn0 = big.tile([128, N], F32R, tag="xn0")
            xn1 = big.tile([128, N], F32R, tag="xn1")
            nc.vector.tensor_mul(xn0[:, :], x0[:, :], sinv[:, :])
            nc.vector.tensor_mul(xn1[:, :], x1[:, :], sinv[:, :])
            xns = [xn0, xn1]

            # ---------------- qkv projection (f32r) ----------------
            q_sb = big.tile([128, N], BF16)
            k_sb = big.tile([128, NK], BF16)
            v_sb = big.tile([128, N], F32)
            dsts = [q_sb, k_sb, v_sb]
            for m in range(3):
                for c0, cw in CHUNKS:
                    qp = ps_a.tile([128, 512], F32, tag="a0")
                    for kt in range(2):
                        nc.tensor.matmul(
                            qp[:, 0:cw],
                            wq[:, kt, m * 128:(m + 1) * 128],
                            xns[kt][:, c0:c0 + cw],
                            start=(kt == 0), stop=(kt == 1),
                        )
                    if m == 1:
                        nc.scalar.copy(dsts[m][:, c0:c0 + cw], qp[:, 0:cw])
                    else:
                        nc.vector.tensor_copy(dsts[m][:, c0:c0 + cw], qp[:, 0:cw])
            # mem keys at k columns 2304:2308
            nc.vector.tensor_copy(k_sb[:, N:NK], memk_f[:, :])

            # ---------------- v^T tiles ----------------
            # per head: [key(128 part), 19 jt, 128]: col 0 ones, 1:64 zeros, 64:128 v
            vTA = big.tile([128, NJT + 1, 128], BF16, tag="vTA")
            vTB = big.tile([128, NJT + 1, 128], BF16, tag="vTB")
            vTs = [vTA, vTB]
            for h in range(2):
                nc.vector.tensor_copy(
                    vTs[h][:, :, 0:1],
                    ones_f[:, :].to_broadcast((128, NJT + 1, 1)))
                nc.vector.tensor_copy(
                    vTs[h][:, :, 1:64],
                    zeros_f[:, :].to_broadcast((128, NJT + 1, 63)))
                for jt in range(NJT):
                    tp = ps_a.tile([128, 64], F32, tag="a1")
                    nc.tensor.transpose(
                        tp[:, :],
                        v_sb[64 * h:64 * h + 64, jt * 128:(jt + 1) * 128],
                        ident[64 * h:64 * h + 64, 64 * h:64 * h + 64],
                    )
                    nc.vector.tensor_copy(vTs[h][:, jt, 64:128], tp[:, :])
                # mem values into the last j-tile (rows 0:4)
                nc.vector.tensor_copy(vTs[h][0:MEM, NJT, 64:128], memv_f[:, h, :])

            # ---------------- attention + out projection ----------------
            rec = io.tile([1, 2, 512], F32, tag="rec")
            for ci, (c0, cw) in enumerate(CHUNKS):
                acc0 = ps_a.tile([128, 512], F32, tag="a0")
                acc1 = ps_a.tile([128, 512], F32, tag="a1")
                accs = [acc0, acc1]
                for jt in range(NJT + 1):
                    if jt < NJT:
                        kk, km = jt * 128, 128
                    else:
                        kk, km = N, MEM
                    s_ps = ps_s.tile([128, 2, 512], F32, tag="s")
                    for h in range(2):
                        nc.tensor.matmul(
                            s_ps[0:km, h, 0:cw],
                            k_sb[64 * h:64 * h + 64, kk:kk + km],
                            q_sb[64 * h:64 * h + 64, c0:c0 + cw],
                            start=True, stop=True,
                        )
                    P = pP.tile([128, 2, 512], BF16, tag="P")
                    nc.scalar.activation(P[0:km, :, 0:cw], s_ps[0:km, :, 0:cw], EXP)
                    for h in range(2):
                        nc.tensor.matmul(
                            accs[h][:, 0:cw],
                            vTs[h][0:km, jt, :],
                            P[0:km, h, 0:cw],
                            start=(jt == 0), stop=(jt == NJT),
                            skip_group_check=True,
                        )
                # normalize: out^T_h = acc[64:128] * (1/acc[0])
                rb = pP.tile([128, 2, 512], F32, tag="rb")
                for h in range(2):
                    nc.vector.reciprocal_approx_fast(
                        rec[0:1, h, 0:cw], accs[h][0:1, 0:cw])
                    nc.gpsimd.partition_broadcast(rb[:, h, 0:cw], rec[0:1, h, 0:cw])
                oT0 = pP.tile([128, 512], BF16, tag="oT0")
                oT1 = pP.tile([128, 512], BF16, tag="oT1")
                oTs = [oT0, oT1]
                for h in range(2):
                    nc.vector.tensor_mul(
                        oTs[h][64:128, 0:cw], accs[h][64:128, 0:cw],
                        rb[64:128, h, 0:cw])
                # out projection: [256, cw] partial = sum_h woutT_h.T @ oT_h
                osb = pP.tile([128, 2, 512], F32, tag="osb")
                for mt in range(2):
                    op = ps_a.tile([128, 512], F32, tag=f"a{mt}")
                    for h in range(2):
                        nc.tensor.matmul(
                            op[:, 0:cw],
                            wouts[h][64:128, mt * 128:(mt + 1) * 128],
                            oTs[h][64:128, 0:cw],
                            start=(h == 0), stop=(h == 1),
                        )
                    nc.vector.tensor_copy(osb[:, mt, 0:cw], op[:, 0:cw])
                # chunked reduce-scatter of the [256, cw] partial
                bi = dram.tile([2, 128, cw], F32, tag=f"bi{ci}")
                bo = dram.tile([DHEAD, cw], F32, tag=f"bo{ci}")
                nc.sync.dma_start(out=bi[0, :, :], in_=osb[:, 0, 0:cw])
                nc.sync.dma_start(out=bi[1, :, :], in_=osb[:, 1, 0:cw])
                nc.gpsimd.collective_compute(
                    "ReduceScatter", mybir.AluOpType.add,
                    replica_groups=GROUPS,
                    ins=[bi[:, :, :].opt()],
                    outs=[bo[:, :].opt()],
                )
                nc.sync.dma_start(out=out_d[:, c0:c0 + cw], in_=bo[:, :])
    nc.compile()
    return nc


_NC = None
_last_in_maps = None


def _get_nc():
    global _NC
    if _NC is None:
        _NC = build()
    return _NC


def make_in_maps(x, gamma, mem_kv, w_qkv, w_out):
    x = np.asarray(x, np.float32)
    gamma = np.asarray(gamma, np.float32).reshape(DIM)
    mem_kv = np.asarray(mem_kv, np.float32)
    w_qkv = np.asarray(w_qkv, np.float32)
    w_out = np.asarray(w_out, np.float32)

    g1 = 1.0 + gamma  # [256]
    scale = DHEAD ** -0.5
    in_maps = []
    for core in range(8):
        b, g = core // 4, core % 4
        hA, hB = 2 * g, 2 * g + 1
        blocks = []
        for t in range(3):  # q, k, v
            for h in (hA, hB):
                wblk = w_qkv[t * HID + h * DHEAD: t * HID + (h + 1) * DHEAD, :]
                if t == 0:
                    wblk = wblk * scale
                blocks.append(wblk.T)  # [256, 64]
        wqkvT = np.concatenate(blocks, axis=1) * g1[:, None]  # [256, 384]
        memk = np.concatenate(
            [mem_kv[0, hA].T, mem_kv[0, hB].T], axis=0)  # [128, 4]
        memv = np.stack([mem_kv[1, hA], mem_kv[1, hB]], axis=1)  # [4, 2, 64]
        woutT = np.stack(
            [w_out[:, hA * DHEAD:(hA + 1) * DHEAD].T,
             w_out[:, hB * DHEAD:(hB + 1) * DHEAD].T], axis=0)  # [2, 64, 256]
        in_maps.append({
            "x": np.ascontiguousarray(x[b].reshape(DIM, N)),
            "wqkv": np.ascontiguousarray(wqkvT),
            "memk": np.ascontiguousarray(memk),
            "memv": np.ascontiguousarray(memv),
            "woutT": np.ascontiguousarray(woutT),
        })
    return in_maps


def kernel(x, gamma, mem_kv, w_qkv, w_out):
    global _last_in_maps
    in_maps = make_in_maps(x, gamma, mem_kv, w_qkv, w_out)
    _last_in_maps = in_maps
    nc = _get_nc()
    res = run_bass_kernel_spmd(nc, in_maps, core_ids=list(range(8)))
    out = np.empty((2, DIM, N), np.float32)
    for core in range(8):
        b, g = core // 4, core % 4
        out[b, 64 * g:64 * g + 64, :] = res.results[core]["out"]
    return out.reshape(2, DIM, 48, 48)
